# revision 1
# baseline (speedup 1.0000x reference)
"""Depthwise 31x31 conv (32,384,56,56) on 8 TRN2 NeuronCores.

Strategy: channel-shard 384 -> 48 per core (depthwise is per-channel
independent, no communication). Per channel, the 2D conv is computed on
the TensorEngine as 32 PSUM-accumulated matmuls:
  stationary lhsT [K=112, M=112]: K = (2 w-shift copies x 56 h_in),
  M = (w-parity x 56 h_out), holding a Toeplitz-over-h arrangement of the
  31x31 taps (host-precomputed table; kw is covered by the d-step index,
  the w-shift copy rc, and the output w-parity wr: kw = 2d + rc - wr).
  moving rhs [112, 448]: 16 images x 28 w-blocks read strided from a
  host-prepadded x layout. Output returns in PSUM-native layout and is
  reassembled on the host.
"""

import sys

sys.path.insert(0, "/opt/trn_rl_repo")

import numpy as np
import concourse.bacc as bacc
import concourse.mybir as mybir
import concourse.tile as tile
from concourse.bass_utils import run_bass_kernel_spmd

H = W = 56
KK = 31
PAD = 15
ND = 16
WS = 86
KP = 112
MP = 112
N_CORES = 8
C_TOTAL = 384
IMGS = 32
C_PER = C_TOTAL // N_CORES

_DT_IN = mybir.dt.float16
_MPAD = 128

_nc_cache = {}


def _host_prepare(x, weight, io_dtype, mpad=MP):
    C = x.shape[1]
    imgs = x.shape[0]
    xpre = np.zeros((C, 2, H, imgs, WS), dtype=io_dtype)
    xc = np.ascontiguousarray(x.transpose(1, 2, 0, 3)).astype(io_dtype)
    for rc in range(2):
        xpre[:, rc, :, :, 15 - rc:15 - rc + W] = xc
    xpre = xpre.reshape(C, KP, imgs, WS)

    w = weight.reshape(C, KK, KK).astype(np.float32)
    wtab = np.zeros((C, 2, H, ND, 2, H), dtype=np.float32)
    hi = np.arange(H)[:, None]
    ho = np.arange(H)[None, :]
    kh = hi - ho + PAD
    khv = (kh >= 0) & (kh < KK)
    khc = np.clip(kh, 0, KK - 1)
    for d in range(ND):
        for rc in range(2):
            for wr in range(2):
                kw = 2 * d + rc - wr
                if not (0 <= kw < KK):
                    continue
                wtab[:, rc, :, d, wr, :] = w[:, khc, kw] * khv[None, :, :]
    wtab = wtab.reshape(C, KP, ND, MP)
    if mpad > MP:
        wtab = np.concatenate(
            [wtab, np.zeros((C, KP, ND, mpad - MP), wtab.dtype)], axis=3)
    return xpre, wtab.astype(io_dtype)


def _build_nc(C, imgs, n_cores, dt_in, repeat=1, mpad=MP, loop_repeat=1,
              d_outer=False):
    f32 = mybir.dt.float32
    is_f32r = dt_in == mybir.dt.float32r
    dt_store = f32 if is_f32r else dt_in

    nc = bacc.Bacc("TRN2", target_bir_lowering=False, debug=False,
                   num_devices=n_cores)
    xp = nc.dram_tensor("xp", [C, KP, imgs, WS], dt_store, kind="ExternalInput")
    wt = nc.dram_tensor("wt", [C, KP, ND, mpad], dt_store, kind="ExternalInput")
    yt = nc.dram_tensor("yt", [C, MP, imgs, W // 2], f32, kind="ExternalOutput")
    xp_ap, wt_ap, yt_ap = xp.ap(), wt.ap(), yt.ap()

    half_sz = 16
    halves = (imgs + half_sz - 1) // half_sz

    with tile.TileContext(nc) as tc:
        with (
            tc.tile_pool(name="xpool", bufs=3) as xpool,
            tc.tile_pool(name="wpool", bufs=3) as wpool,
            tc.tile_pool(name="ypool", bufs=3) as ypool,
            tc.tile_pool(name="psum", bufs=4, space="PSUM") as psum,
        ):
            def body(_iv=None):
                for c in [ci for _ in range(repeat) for ci in range(C)]:
                    xt = xpool.tile([KP, imgs, WS], dt_store)
                    nc.sync.dma_start(xt[:], xp_ap[c])
                    wtt = wpool.tile([KP, ND, mpad], dt_store)
                    nc.sync.dma_start(wtt[:], wt_ap[c])
                    ytile = ypool.tile([MP, imgs, W // 2], f32)
                    bounds = [(half_sz * hf, min(imgs, half_sz * hf + half_sz))
                              for hf in range(halves)]

                    def mm(ps, d, i0, i1):
                        lhsT = wtt[:, d, :]
                        rhs = xt[:, i0:i1, 2 * d: 2 * d + W: 2]
                        if is_f32r:
                            lhsT = lhsT.bitcast(dt_in)
                            rhs = rhs.bitcast(dt_in)
                        nc.tensor.matmul(ps[:], lhsT, rhs,
                                         start=(d == 0), stop=(d == ND - 1))

                    if d_outer:
                        tiles = [psum.tile([mpad, i1 - i0, W // 2], f32,
                                           name=f"ps{hf}", tag=f"ps{hf}")
                                 for hf, (i0, i1) in enumerate(bounds)]
                        for d in range(ND):
                            for ps, (i0, i1) in zip(tiles, bounds):
                                mm(ps, d, i0, i1)
                        for ps, (i0, i1) in zip(tiles, bounds):
                            nc.vector.tensor_copy(ytile[:, i0:i1, :],
                                                  ps[:MP, :, :])
                    else:
                        for (i0, i1) in bounds:
                            ps = psum.tile([mpad, i1 - i0, W // 2], f32)
                            for d in range(ND):
                                mm(ps, d, i0, i1)
                            nc.vector.tensor_copy(ytile[:, i0:i1, :],
                                                  ps[:MP, :, :])
                    nc.sync.dma_start(yt_ap[c], ytile[:])

            if loop_repeat > 1:
                with tc.For_i(0, loop_repeat, 1):
                    body()
            else:
                body()
    nc.compile()
    return nc


def _get_nc():
    key = (C_PER, IMGS, N_CORES, _DT_IN)
    if key not in _nc_cache:
        _nc_cache[key] = _build_nc(*key, mpad=_MPAD)
    return _nc_cache[key]


def kernel(x, weight, bias_term):
    x = np.asarray(x, dtype=np.float32)
    weight = np.asarray(weight, dtype=np.float32)
    bias_term = np.asarray(bias_term, dtype=np.float32)

    nc = _get_nc()
    np_dt = mybir.dt.np(mybir.dt.float32 if _DT_IN == mybir.dt.float32r
                        else _DT_IN)
    xpre, wtab = _host_prepare(x, weight, np_dt, mpad=_MPAD)

    in_maps = []
    for k in range(N_CORES):
        c0 = k * C_PER
        in_maps.append({
            "xp": np.ascontiguousarray(xpre[c0:c0 + C_PER]),
            "wt": np.ascontiguousarray(wtab[c0:c0 + C_PER]),
        })
    res = run_bass_kernel_spmd(nc, in_maps, list(range(N_CORES)))

    y = np.empty((IMGS, C_TOTAL, H, W), np.float32)
    for k in range(N_CORES):
        c0 = k * C_PER
        yt = res.results[k]["yt"]                      # [C_PER, 112, IMGS, 28]
        yk = yt.reshape(C_PER, 2, H, IMGS, W // 2)     # [c, wr, h, img, bw]
        # -> [img, c, h, bw, wr] -> [img, c, h, w]
        y[:, c0:c0 + C_PER] = yk.transpose(3, 0, 2, 4, 1).reshape(
            IMGS, C_PER, H, W)
    y += bias_term[None, :, None, None]
    return y



# revision 2
# speedup vs baseline: 165.0580x; 165.0580x over previous
"""Depthwise 31x31 conv (32,384,56,56) on 8 TRN2 NeuronCores — v2.

Channel-shard 384 -> 48 per core. Per channel the conv runs on the
TensorEngine as 16 d-steps of PSUM-accumulated matmuls with stationary
lhsT [K=112, M=112] holding a Toeplitz-over-h tap table (kw = 2d+rc-wr)
and moving rhs [112, 448] read strided from a host-prepadded x layout.

v2 over baseline:
  - the per-channel [112,16,112] tap table is expanded ON DEVICE by a
    single overlapping-window DMA from a compact [2,32,111] line buffer
    (HBM weight traffic 458KB -> 14KB per channel).
  - d-steps iterate outermost over the two image halves, so each
    stationary load is reused for both halves (16 loads, 32 matmuls).
  - outputs leave the core as f16 (half the output HBM traffic); the
    host converts to f32 and adds the bias.
"""

import sys

sys.path.insert(0, "/opt/trn_rl_repo")

import numpy as np
import concourse.bacc as bacc
import concourse.mybir as mybir
import concourse.tile as tile
from concourse.ap import AP
from concourse.bass_utils import run_bass_kernel_spmd

H = W = 56
KK = 31
PAD = 15
ND = 16
WS = 86
KP = 112
MP = 112
N_CORES = 8
C_TOTAL = 384
IMGS = 32
C_PER = C_TOTAL // N_CORES
WL = 111
DW = 2 * ND

_nc_cache = {}


def _host_prepare(x, weight):
    C = x.shape[1]
    imgs = x.shape[0]
    xp = np.zeros((C, 2, H, imgs, WS), dtype=np.float16)
    xc = np.ascontiguousarray(x.transpose(1, 2, 0, 3)).astype(np.float16)
    for rc in range(2):
        xp[:, rc, :, :, 15 - rc:15 - rc + W] = xc
    xp = xp.reshape(C, KP, imgs, WS)

    # Compact Toeplitz lines: ws[c, rc, dw, j] with dw = 2d + wr encodes
    # the [56,56] (h_in, h_out) tap block for kw = 2d + rc - wr. The
    # stationary is laid out as a HANKEL matrix (h_out stored reversed:
    # h_o' = 55 - h_o) so the sliding-window DMA expansion
    #   block[h_i, h_o'] = ws[..., h_i + h_o']
    # has positive unit strides in both dims (DMA APs require the final
    # dim contiguous ascending). ws[..., j] = w[c, j - 40, kw] for
    # j in [40, 71), zero elsewhere; the host un-reverses h_o when
    # reassembling the output.
    w = weight.reshape(C, KK, KK).astype(np.float32)
    ws = np.zeros((C, 2, DW, WL), dtype=np.float16)
    for rc in range(2):
        for dw in range(DW):
            d, wr = dw >> 1, dw & 1
            kw = 2 * d + rc - wr
            if 0 <= kw < KK:
                ws[:, rc, dw, 40:71] = w[:, :, kw]
    return xp, ws


def _build_nc(C, imgs, n_cores, loop_repeat=1):
    f16, f32 = mybir.dt.float16, mybir.dt.float32
    nc = bacc.Bacc("TRN2", target_bir_lowering=False, debug=False,
                   num_devices=n_cores)
    xp = nc.dram_tensor("xp", [C, KP, imgs, WS], f16, kind="ExternalInput")
    ws = nc.dram_tensor("ws", [C, 2, DW, WL], f16, kind="ExternalInput")
    yt = nc.dram_tensor("yt", [C, MP, imgs, W // 2], f16,
                        kind="ExternalOutput")
    xp_ap, ws_ap, yt_ap = xp.ap(), ws.ap(), yt.ap()
    half = imgs // 2

    with tile.TileContext(nc) as tc:
        with (
            tc.tile_pool(name="xpool", bufs=3) as xpool,
            tc.tile_pool(name="wpool", bufs=3) as wpool,
            tc.tile_pool(name="ypool", bufs=3) as ypool,
            tc.tile_pool(name="psum", bufs=4, space="PSUM") as psum,
        ):
            def body(_iv=None):
                for c in range(C):
                    xt = xpool.tile([KP, imgs, WS], f16)
                    nc.sync.dma_start(xt[:], xp_ap[c])
                    wt = wpool.tile([KP, ND, MP], f16)
                    # Overlapping-window expansion: partition (rc, h_i),
                    # free (dw, h_o) reads ws[c, rc, dw, h_o - h_i + 55].
                    # One DMA per rc half (DMA APs are limited to 3 dims).
                    for rc in range(2):
                        src = AP(tensor=ws_ap.tensor,
                                 offset=(c * 2 + rc) * (DW * WL),
                                 ap=[[1, H], [WL, DW], [1, H]])
                        nc.sync.dma_start(wt[rc * H:(rc + 1) * H], src)
                    ps = [psum.tile([MP, half, W // 2], f32,
                                    name=f"ps{hf}", tag=f"ps{hf}")
                          for hf in range(2)]
                    for d in range(ND):
                        lhsT = wt[:, d, :]
                        for hf in range(2):
                            i0 = hf * half
                            rhs = xt[:, i0:i0 + half, 2 * d: 2 * d + W: 2]
                            nc.tensor.matmul(ps[hf][:], lhsT, rhs,
                                             start=(d == 0),
                                             stop=(d == ND - 1))
                    ytl = ypool.tile([MP, imgs, W // 2], f16)
                    for hf in range(2):
                        i0 = hf * half
                        nc.vector.tensor_copy(ytl[:, i0:i0 + half, :],
                                              ps[hf][:])
                    nc.sync.dma_start(yt_ap[c], ytl[:])

            if loop_repeat > 1:
                with tc.For_i(0, loop_repeat, 1):
                    body()
            else:
                body()
    nc.compile()
    return nc


def _get_nc(loop_repeat=1):
    key = (C_PER, IMGS, N_CORES, loop_repeat)
    if key not in _nc_cache:
        _nc_cache[key] = _build_nc(C_PER, IMGS, N_CORES,
                                   loop_repeat=loop_repeat)
    return _nc_cache[key]


def kernel(x, weight, bias_term):
    x = np.asarray(x, dtype=np.float32)
    weight = np.asarray(weight, dtype=np.float32)
    bias_term = np.asarray(bias_term, dtype=np.float32)

    nc = _get_nc()
    xp, ws = _host_prepare(x, weight)

    in_maps = []
    for k in range(N_CORES):
        c0 = k * C_PER
        in_maps.append({
            "xp": np.ascontiguousarray(xp[c0:c0 + C_PER]),
            "ws": np.ascontiguousarray(ws[c0:c0 + C_PER]),
        })
    res = run_bass_kernel_spmd(nc, in_maps, list(range(N_CORES)))

    y = np.empty((IMGS, C_TOTAL, H, W), np.float32)
    for k in range(N_CORES):
        c0 = k * C_PER
        yk = res.results[k]["yt"].astype(np.float32)
        yk = yk.reshape(C_PER, 2, H, IMGS, W // 2)[:, :, ::-1]
        y[:, c0:c0 + C_PER] = yk.transpose(3, 0, 2, 4, 1).reshape(
            IMGS, C_PER, H, W)
    y += bias_term[None, :, None, None]
    return y


# revision 3
# speedup vs baseline: 169.7696x; 1.0285x over previous
"""Depthwise 31x31 conv (32,384,56,56) on 8 TRN2 NeuronCores — v2.

Channel-shard 384 -> 48 per core. Per channel the conv runs on the
TensorEngine as 16 d-steps of PSUM-accumulated matmuls with stationary
lhsT [K=112, M=112] holding a Toeplitz-over-h tap table (kw = 2d+rc-wr)
and moving rhs [112, 448] read strided from a host-prepadded x layout.

v2 over baseline:
  - the per-channel [112,16,112] tap table is expanded ON DEVICE by a
    single overlapping-window DMA from a compact [2,32,111] line buffer
    (HBM weight traffic 458KB -> 14KB per channel).
  - d-steps iterate outermost over the two image halves, so each
    stationary load is reused for both halves (16 loads, 32 matmuls).
  - outputs leave the core as f16 (half the output HBM traffic); the
    host converts to f32 and adds the bias.
"""

import sys

sys.path.insert(0, "/opt/trn_rl_repo")

import numpy as np
import concourse.bacc as bacc
import concourse.mybir as mybir
import concourse.tile as tile
from concourse.ap import AP
from concourse.bass_utils import run_bass_kernel_spmd

H = W = 56
KK = 31
PAD = 15
ND = 16
WS = 86
KP = 112
MP = 112
N_CORES = 8
C_TOTAL = 384
IMGS = 32
C_PER = C_TOTAL // N_CORES
WL = 111
DW = 2 * ND

_nc_cache = {}


def _host_prepare(x, weight):
    C = x.shape[1]
    imgs = x.shape[0]
    xp = np.zeros((C, 2, H, imgs, WS), dtype=np.float16)
    xc = np.ascontiguousarray(x.transpose(1, 2, 0, 3)).astype(np.float16)
    for rc in range(2):
        xp[:, rc, :, :, 15 - rc:15 - rc + W] = xc
    xp = xp.reshape(C, KP, imgs, WS)

    # Compact Toeplitz lines: ws[c, rc, dw, j] with dw = 2d + wr encodes
    # the [56,56] (h_in, h_out) tap block for kw = 2d + rc - wr. The
    # stationary is laid out as a HANKEL matrix (h_out stored reversed:
    # h_o' = 55 - h_o) so the sliding-window DMA expansion
    #   block[h_i, h_o'] = ws[..., h_i + h_o']
    # has positive unit strides in both dims (DMA APs require the final
    # dim contiguous ascending). ws[..., j] = w[c, j - 40, kw] for
    # j in [40, 71), zero elsewhere; the host un-reverses h_o when
    # reassembling the output.
    w = weight.reshape(C, KK, KK).astype(np.float32)
    ws = np.zeros((C, 2, DW, WL), dtype=np.float16)
    for rc in range(2):
        for dw in range(DW):
            d, wr = dw >> 1, dw & 1
            kw = 2 * d + rc - wr
            if 0 <= kw < KK:
                ws[:, rc, dw, 40:71] = w[:, :, kw]
    return xp, ws


def _build_nc(C, imgs, n_cores, loop_repeat=1):
    f16, f32 = mybir.dt.float16, mybir.dt.float32
    nc = bacc.Bacc("TRN2", target_bir_lowering=False, debug=False,
                   num_devices=n_cores)
    xp = nc.dram_tensor("xp", [C, KP, imgs, WS], f16, kind="ExternalInput")
    ws = nc.dram_tensor("ws", [C, 2, DW, WL], f16, kind="ExternalInput")
    yt = nc.dram_tensor("yt", [C, MP, imgs, W // 2], f16,
                        kind="ExternalOutput")
    xp_ap, ws_ap, yt_ap = xp.ap(), ws.ap(), yt.ap()
    half = imgs // 2

    with tile.TileContext(nc) as tc:
        with (
            tc.tile_pool(name="xpool", bufs=3) as xpool,
            tc.tile_pool(name="wpool", bufs=3) as wpool,
            tc.tile_pool(name="ypool", bufs=3) as ypool,
            tc.tile_pool(name="psum", bufs=4, space="PSUM") as psum,
        ):
            def body(_iv=None):
                for c in range(C):
                    xt = xpool.tile([KP, imgs, WS], f16)
                    nc.sync.dma_start(xt[:], xp_ap[c])
                    wt = wpool.tile([KP, ND, MP], f16)
                    # Overlapping-window expansion: partition (rc, h_i),
                    # free (dw, h_o) reads ws[c, rc, dw, h_o - h_i + 55].
                    # One DMA per rc half (DMA APs are limited to 3 dims).
                    for rc in range(2):
                        src = AP(tensor=ws_ap.tensor,
                                 offset=(c * 2 + rc) * (DW * WL),
                                 ap=[[1, H], [WL, DW], [1, H]])
                        nc.scalar.dma_start(wt[rc * H:(rc + 1) * H], src)
                    ps = [psum.tile([MP, half, W // 2], f32,
                                    name=f"ps{hf}", tag=f"ps{hf}")
                          for hf in range(2)]
                    for d in range(ND):
                        lhsT = wt[:, d, :]
                        for hf in range(2):
                            i0 = hf * half
                            rhs = xt[:, i0:i0 + half, 2 * d: 2 * d + W: 2]
                            nc.tensor.matmul(ps[hf][:], lhsT, rhs,
                                             start=(d == 0),
                                             stop=(d == ND - 1))
                    ytl = ypool.tile([MP, imgs, W // 2], f16)
                    for hf in range(2):
                        i0 = hf * half
                        nc.vector.tensor_copy(ytl[:, i0:i0 + half, :],
                                              ps[hf][:])
                    nc.scalar.dma_start(yt_ap[c], ytl[:])

            if loop_repeat > 1:
                with tc.For_i(0, loop_repeat, 1):
                    body()
            else:
                body()
    nc.compile()
    return nc


def _get_nc(loop_repeat=1):
    key = (C_PER, IMGS, N_CORES, loop_repeat)
    if key not in _nc_cache:
        _nc_cache[key] = _build_nc(C_PER, IMGS, N_CORES,
                                   loop_repeat=loop_repeat)
    return _nc_cache[key]


def kernel(x, weight, bias_term):
    x = np.asarray(x, dtype=np.float32)
    weight = np.asarray(weight, dtype=np.float32)
    bias_term = np.asarray(bias_term, dtype=np.float32)

    nc = _get_nc()
    xp, ws = _host_prepare(x, weight)

    in_maps = []
    for k in range(N_CORES):
        c0 = k * C_PER
        in_maps.append({
            "xp": np.ascontiguousarray(xp[c0:c0 + C_PER]),
            "ws": np.ascontiguousarray(ws[c0:c0 + C_PER]),
        })
    res = run_bass_kernel_spmd(nc, in_maps, list(range(N_CORES)))

    y = np.empty((IMGS, C_TOTAL, H, W), np.float32)
    for k in range(N_CORES):
        c0 = k * C_PER
        yk = res.results[k]["yt"].astype(np.float32)
        yk = yk.reshape(C_PER, 2, H, IMGS, W // 2)[:, :, ::-1]
        y[:, c0:c0 + C_PER] = yk.transpose(3, 0, 2, 4, 1).reshape(
            IMGS, C_PER, H, W)
    y += bias_term[None, :, None, None]
    return y
